# revision 10
# baseline (speedup 1.0000x reference)
"""Trainium2 Bass kernel for ConsecutiveMatch (pairwise-cosine top-1 retrieval).

Problem: desc_src [8,2048,256], desc_dst [8,2048,256], points_dst [8,2048,2].
  sim = cos_sim(desc_src, desc_dst) per batch -> [2048, 2048]
  confidence = max_m sim;  idx = argmax_m sim;  matched = points_dst[idx]
Returns (matched [8,2048,2] f32, confidence [8,2048] f32), like the reference.

Sharding: data-parallel over batch B=8 across the 8 NeuronCores (one batch
per core, SPMD: same program, different input tensors per core).

Per-core algorithm:
  - dst descriptors are normalized (rows scaled by 1/||dst_m||, Newton-refined
    rsqrt so the scale is fp32-accurate) BEFORE the matmul; src norms are
    applied to the row max at the END (a positive per-row scale does not
    change the argmax).
  - PE computes sim row-blocks [128 x 2048] fp32 into 4 PSUM banks.
  - DVE computes two strided max-reductions per row-block:
       bmax[p, b] = max_c sim[p, 128*b + c]   (16 block maxes)
       cmax[p, c] = max_b sim[p, 128*b + c]   (128 column maxes)
    For a unique row max g: the first b with bmax[p,b]==g and the first c
    with cmax[p,c]==g factorize argmax = 128*b + c exactly (max8+max_index).
  - matched points are gathered on-device via indirect DMA from points_dst.
"""

import sys

if "/opt/trn_rl_repo" not in sys.path:
    sys.path.insert(0, "/opt/trn_rl_repo")

import numpy as np

P = 128
N = 2048
M = 2048
D = 256
NB = N // P      # 16 row blocks
MBLK = M // P    # 16 col blocks for bmax
KC = D // P      # 2 contraction chunks
BANK = 512
NBANKS = M // BANK  # 4

_PROGRAM = None


def _emit(nc, tc, aps, batched_gather=False):
    import concourse.bass as bass
    import concourse.mybir as mybir
    from concourse.masks import make_identity

    dt = mybir.dt
    Alu = mybir.AluOpType
    Ax = mybir.AxisListType
    src_d, dst_d, pts_d, matched_d, conf_d, idx_d = aps

    with (
        tc.tile_pool(name="persist", bufs=1) as persist,
        tc.tile_pool(name="work", bufs=4) as work,
        tc.tile_pool(name="small", bufs=4) as small,
    ):
        identity = persist.tile([P, P], dt.float32)
        make_identity(nc, identity[:])

        dstT = [
            [persist.tile([P, BANK], dt.float32, name=f"dstT{k}_{b}") for b in range(NBANKS)]
            for k in range(KC)
        ]
        srcT = [
            [persist.tile([P, P], dt.float32, name=f"srcT{j}_{k}") for k in range(KC)]
            for j in range(NB)
        ]
        dst_nat = [persist.tile([P, D], dt.float32, name=f"dst_nat{i}") for i in range(MBLK)]
        src_nat = [persist.tile([P, D], dt.float32, name=f"src_nat{j}") for j in range(NB)]
        ss_dst = persist.tile([P, MBLK], dt.float32)
        ss_src = persist.tile([P, NB], dt.float32)
        gmax_all = persist.tile([P, NB], dt.float32)
        blk_all = persist.tile([P, NB], dt.uint32)
        off_all = persist.tile([P, NB], dt.uint32)

        def newton_rsqrt(ss, n, tag):
            """rn = 1/sqrt(ss) [P, n], ACT-sqrt seed + one Newton step."""
            nrm = small.tile([P, n], dt.float32, name=f"nrm_{tag}")
            nc.scalar.sqrt(nrm[:], ss[:])
            y0 = small.tile([P, n], dt.float32, name=f"y0_{tag}")
            nc.vector.reciprocal(y0[:], nrm[:])
            y0sq = small.tile([P, n], dt.float32, name=f"y0sq_{tag}")
            nc.vector.tensor_mul(y0sq[:], y0[:], y0[:])
            t1 = small.tile([P, n], dt.float32, name=f"t1_{tag}")
            nc.vector.tensor_mul(t1[:], ss[:], y0sq[:])
            t2 = small.tile([P, n], dt.float32, name=f"t2_{tag}")
            nc.vector.tensor_scalar(
                out=t2[:], in0=t1[:], scalar1=-0.5, scalar2=1.5,
                op0=Alu.mult, op1=Alu.add,
            )
            rn = small.tile([P, n], dt.float32, name=f"rn_{tag}")
            nc.vector.tensor_mul(rn[:], y0[:], t2[:])
            return rn

        # ---- dst prep ----
        for i in range(MBLK):
            nc.sync.dma_start(dst_nat[i][:], dst_d[i * P : (i + 1) * P, :])
        for i in range(MBLK):
            sqjunk = work.tile([P, D], dt.float32, name="sqjunk")
            nc.scalar.activation(
                sqjunk[:], dst_nat[i][:], mybir.ActivationFunctionType.Square,
                accum_out=ss_dst[:, i : i + 1],
            )
        rn_dst = newton_rsqrt(ss_dst, MBLK, "d")
        with tc.tile_pool(name="psum_tp", bufs=4, space="PSUM") as psum_tp:
            for i in range(MBLK):
                dhat = work.tile([P, D], dt.float32, name="dhat")
                nc.vector.tensor_scalar_mul(dhat[:], dst_nat[i][:], rn_dst[:, i : i + 1])
                for k in range(KC):
                    pt = psum_tp.tile([P, P], dt.float32, name="pt")
                    nc.tensor.transpose(pt[:], dhat[:, k * P : (k + 1) * P], identity[:])
                    dest = dstT[k][i // 4][:, (i % 4) * P : (i % 4 + 1) * P]
                    if (i + k) % 2 == 0:
                        nc.scalar.copy(dest, pt[:])
                    else:
                        nc.vector.tensor_copy(dest, pt[:])

        # ---- src loads (queued upfront; consumed per main-loop block) ----
        for j in range(NB):
            nc.sync.dma_start(src_nat[j][:], src_d[j * P : (j + 1) * P, :])

        # ---- main loop ----
        with tc.tile_pool(name="psum_sim", bufs=2, space="PSUM") as psum_sim:
            for j in range(NB):
                ps = psum_sim.tile([P, M], dt.float32, name="ps")
                # src norms + transpose, borrowing bank0 of ps before matmuls
                sqj2 = work.tile([P, D], dt.float32, name="sqj2")
                nc.scalar.activation(
                    sqj2[:], src_nat[j][:], mybir.ActivationFunctionType.Square,
                    accum_out=ss_src[:, j : j + 1],
                )
                for k in range(KC):
                    pview = ps[:, k * P : (k + 1) * P]
                    nc.tensor.transpose(pview, src_nat[j][:, k * P : (k + 1) * P], identity[:])
                    nc.scalar.copy(srcT[j][k][:], pview)
                for k in range(KC):
                    for mb in range(NBANKS):
                        nc.tensor.matmul(
                            ps[:, mb * BANK : (mb + 1) * BANK],
                            lhsT=srcT[j][k][:],
                            rhs=dstT[k][mb][:],
                            start=(k == 0),
                            stop=(k == KC - 1),
                        )
                bmax = small.tile([P, MBLK], dt.float32, name="bmax")
                nc.vector.reduce_max(
                    out=bmax[:],
                    in_=ps[:].rearrange("p (b c) -> p b c", c=P),
                    axis=Ax.X,
                )
                cmax = work.tile([P, P], dt.float32, name="cmax")
                nc.vector.reduce_max(
                    out=cmax[:],
                    in_=ps[:].rearrange("p (b c) -> p c b", c=P),
                    axis=Ax.X,
                )
                gmax8 = small.tile([P, 8], dt.float32, name="gmax8")
                nc.vector.max(out=gmax8[:], in_=bmax[:])
                blk8 = small.tile([P, 8], dt.uint32, name="blk8")
                nc.vector.max_index(out=blk8[:], in_max=gmax8[:], in_values=bmax[:])
                off8 = small.tile([P, 8], dt.uint32, name="off8")
                nc.vector.max_index(out=off8[:], in_max=gmax8[:], in_values=cmax[:])
                nc.scalar.copy(gmax_all[:, j : j + 1], gmax8[:, 0:1])
                nc.scalar.copy(blk_all[:, j : j + 1], blk8[:, 0:1])
                nc.scalar.copy(off_all[:, j : j + 1], off8[:, 0:1])

        # ---- epilogue (batched) ----
        rn_src = newton_rsqrt(ss_src, NB, "s")
        conf_all = persist.tile([P, NB], dt.float32)
        nc.vector.tensor_mul(conf_all[:], gmax_all[:], rn_src[:])

        blkf = small.tile([P, NB], dt.float32, name="blkf")
        nc.vector.tensor_copy(blkf[:], blk_all[:])
        offf = small.tile([P, NB], dt.float32, name="offf")
        nc.vector.tensor_copy(offf[:], off_all[:])
        idxf = small.tile([P, NB], dt.float32, name="idxf")
        nc.vector.tensor_scalar(
            out=idxf[:], in0=blkf[:], scalar1=float(P), scalar2=None, op0=Alu.mult,
        )
        idxf2 = small.tile([P, NB], dt.float32, name="idxf2")
        nc.vector.tensor_add(idxf2[:], idxf[:], offf[:])
        idxi_all = persist.tile([P, NB], dt.int32)
        nc.vector.tensor_copy(idxi_all[:], idxf2[:])

        # conf[n], idx[n] with n = j*128 + p: partition stride 1, j stride 128
        nc.sync.dma_start(bass.AP(conf_d, 0, [[1, P], [P, NB]]), conf_all[:])
        nc.sync.dma_start(bass.AP(idx_d, 0, [[1, P], [P, NB]]), idxi_all[:])

        mt_all = persist.tile([P, NB, 2], dt.float32)
        if batched_gather:
            nc.gpsimd.indirect_dma_start(
                out=mt_all[:],
                out_offset=None,
                in_=pts_d[:],
                in_offset=bass.IndirectOffsetOnAxis(ap=idxi_all[:], axis=0),
            )
        else:
            for j in range(NB):
                nc.gpsimd.indirect_dma_start(
                    out=mt_all[:, j],
                    out_offset=None,
                    in_=pts_d[:],
                    in_offset=bass.IndirectOffsetOnAxis(ap=idxi_all[:, j : j + 1], axis=0),
                )
        # matched[(j*128+p), c]: partition p stride 2, j stride 256, c stride 1
        nc.sync.dma_start(
            bass.AP(matched_d, 0, [[2, P], [P * 2, NB], [1, 2]]), mt_all[:]
        )


def _build_program(repeat=1, batched_gather=False):
    import concourse.mybir as mybir
    import concourse.tile as tile
    from concourse import bacc

    dt = mybir.dt
    nc = bacc.Bacc("TRN2", target_bir_lowering=False, debug=False)

    src_d = nc.dram_tensor("desc_src", [N, D], dt.float32, kind="ExternalInput").ap()
    dst_d = nc.dram_tensor("desc_dst", [M, D], dt.float32, kind="ExternalInput").ap()
    pts_d = nc.dram_tensor("points_dst", [M, 2], dt.float32, kind="ExternalInput").ap()
    matched_d = nc.dram_tensor("matched", [N, 2], dt.float32, kind="ExternalOutput")
    conf_d = nc.dram_tensor("conf", [N], dt.float32, kind="ExternalOutput")
    idx_d = nc.dram_tensor("idx", [N], dt.int32, kind="ExternalOutput")

    aps = (src_d, dst_d, pts_d, matched_d, conf_d, idx_d)
    with tile.TileContext(nc) as tc:
        for _ in range(repeat):
            _emit(nc, tc, aps, batched_gather=batched_gather)
    nc.compile()
    return nc


def get_program():
    global _PROGRAM
    if _PROGRAM is None:
        _PROGRAM = _build_program()
    return _PROGRAM


def run_cores(in_maps, trace=False, **kwargs):
    from concourse.bass_utils import run_bass_kernel_spmd

    nc = get_program()
    return run_bass_kernel_spmd(
        nc, in_maps, core_ids=list(range(len(in_maps))), trace=trace, **kwargs
    )


def kernel(desc_src, desc_dst, points_dst):
    desc_src = np.ascontiguousarray(np.asarray(desc_src, dtype=np.float32))
    desc_dst = np.ascontiguousarray(np.asarray(desc_dst, dtype=np.float32))
    points_dst = np.ascontiguousarray(np.asarray(points_dst, dtype=np.float32))
    B = desc_src.shape[0]
    assert desc_src.shape == (B, N, D) and desc_dst.shape == (B, M, D)
    in_maps = [
        {
            "desc_src": desc_src[b],
            "desc_dst": desc_dst[b],
            "points_dst": points_dst[b],
        }
        for b in range(B)
    ]
    res = run_cores(in_maps)
    matched = np.stack([res.results[b]["matched"] for b in range(B)])
    conf = np.stack([res.results[b]["conf"] for b in range(B)])
    return matched, conf


if __name__ == "__main__":
    rng = np.random.default_rng(0)
    inputs = {
        "desc_src": rng.standard_normal((8, N, D), dtype=np.float32),
        "desc_dst": rng.standard_normal((8, M, D), dtype=np.float32),
        "points_dst": rng.random((8, M, 2), dtype=np.float32),
    }
    matched, conf = kernel(**inputs)
    print("matched", matched.shape, matched.dtype, "conf", conf.shape, conf.dtype)
